# revision 15
# baseline (speedup 1.0000x reference)
"""AGREE group-recommendation kernel for TRN2 (8 cores, data-parallel over groups).

v2: bf16 member gather directly in transposed layout via dma_gather
(transpose=True) on 2 SWDGE queues; bf16 PE matmuls (4x fp32 rate); item
projection folded into the proj matmul via broadcast rhs; per-supertile mask
DMA; bf16 DVE ops.

Per core: 1024 groups x 32 members.
  mT [E=128, 128*L] bf16   <- one transposed gather per supertile (queue s%2)
  hid[0:16] = relu(W1a.T@mT + W1b.T@iT_bcast + b1)   (PSUM accum + ACT relu)
  hid[16]   = mask row (one DMA per supertile)
  s_rep [128, crows] = [W2;1].T @ hid                (bf16 matmul)
  ex = exp(s_rep) bf16; den/pooled via DVE windowed reduces
  g = pooled/den + gT; out = sigmoid(relu([g*i, g, i]@P1 + p1)@P2 + p2) (f32)
"""
import numpy as np
import ml_dtypes

import concourse.bass as bass
import concourse.mybir as mybir
import concourse.tile as tile
from concourse import bacc

F32 = mybir.dt.float32
BF16 = mybir.dt.bfloat16
I32 = mybir.dt.int32
I16 = mybir.dt.int16
AF = mybir.ActivationFunctionType
OP = mybir.AluOpType

B_L = 1024      # groups per core
M = 32          # members per group
E = 128
NST = 8         # supertiles per core
ST_G = 128      # groups per supertile
CH = 1024       # max rows per chunk
NU = 200000
NI = 50000
NG = 20000
MASK_NEG = -40.0
NQ = 2          # SWDGE queues for member gathers
CAPS = (32, 30, 27, 24, 21, 17, 14, 10)  # fallback; data_caps used at runtime

# bf16 reduce outputs: no DVE speedup (TensorReduce has no 2x mode), keep f32
LOWP_REDUCE = False
# fold item projection into proj matmul via broadcast rhs -- flip if unsupported
IP_FOLD = True


def lane_split(caps):  # compat stub for test.py print
    return tuple((L, 0) for L in caps)


def build_kernel(num_devices=8, loop_K=0, caps=CAPS):
    nc = bacc.Bacc("TRN2", target_bir_lowering=False, debug=False,
                   num_devices=num_devices, num_swdge_queues=NQ)
    ap = {}
    def dram(name, shape, dt, kind="ExternalInput"):
        ap[name] = nc.dram_tensor(name, shape, dt, kind=kind).ap()
        return ap[name]

    tot_t = sum(caps)
    tot_w = sum(512 * ((ST_G + min(512 // L, ST_G) - 1) // min(512 // L, ST_G))
                for L in caps)     # padded mask cols
    user = dram("user_emb", [NU, E], BF16)
    gtab = dram("group_emb", [NG, E], BF16)
    itab = dram("item_emb", [NI, E], BF16)
    midx16 = dram("midx16", [128, 8 * tot_t], I16)
    gidx16 = dram("gidx16", [128, B_L // 16], I16)
    iidx16 = dram("iidx16", [128, B_L // 16], I16)
    maskb = dram("maskb", [1, tot_w], BF16)
    w1a = dram("W1a", [E, 16], BF16)
    w1b = dram("W1b", [E, 16], BF16)
    w2m = dram("W2m", [17, 128], BF16)   # [tile(W2,128); ones]
    b1p = dram("b1p", [16, 1], F32)
    p1a = dram("P1a", [E, 16], F32)
    p1b = dram("P1b", [E, 16], F32)
    p1c = dram("P1c", [E, 16], F32)
    p1v = dram("p1v", [16, 1], F32)
    p2m = dram("P2", [16, 1], F32)
    p2v = dram("p2v", [1, 1], F32)
    out = dram("out", [1, B_L], F32, kind="ExternalOutput")

    with tile.TileContext(nc) as tc:
        with (
            tc.tile_pool(name="cst", bufs=1) as cst,
            tc.tile_pool(name="mT", bufs=3) as mTp,
            tc.tile_pool(name="ex", bufs=3) as exp_,
            tc.tile_pool(name="sml", bufs=2) as smlp,
            tc.tile_pool(name="hps", bufs=2, space="PSUM") as hpsp,
            tc.tile_pool(name="srp", bufs=2, space="PSUM") as srpp,
        ):
            # ---- constants ----
            def cload(name, shape, dt):
                t = cst.tile(shape, dt, tag=name)
                nc.sync.dma_start(out=t[:], in_=ap[name][:])
                return t

            midx16_sb = cload("midx16", [128, 8 * tot_t], I16)
            gidx16_sb = cload("gidx16", [128, B_L // 16], I16)
            iidx16_sb = cload("iidx16", [128, B_L // 16], I16)
            w1a_sb = cload("W1a", [E, 16], BF16)
            w1b_sb = cload("W1b", [E, 16], BF16)
            w2m_sb = cload("W2m", [17, 128], BF16)
            b1p_sb = cload("b1p", [16, 1], F32)
            p1a_sb = cload("P1a", [E, 16], F32)
            p1b_sb = cload("P1b", [E, 16], F32)
            p1c_sb = cload("P1c", [E, 16], F32)
            p1_sb = cload("p1v", [16, 1], F32)
            p2m_sb = cload("P2", [16, 1], F32)
            p2v_sb = cload("p2v", [1, 1], F32)

            # group/item embeddings: transposed bf16 gathers -> f32 copies
            gT = cst.tile([128, 1, B_L], BF16, tag="gT")
            iT = cst.tile([128, 1, B_L], BF16, tag="iT")
            nc.gpsimd.dma_gather(
                out_ap=gT[:, :, :], in_ap=gtab[:], idxs_ap=gidx16_sb[:],
                num_idxs=B_L, num_idxs_reg=B_L, elem_size=E,
                transpose=True, single_packet=False, queue_num=0)
            nc.gpsimd.dma_gather(
                out_ap=iT[:, :, :], in_ap=itab[:], idxs_ap=iidx16_sb[:],
                num_idxs=B_L, num_idxs_reg=B_L, elem_size=E,
                transpose=True, single_packet=False, queue_num=1 % NQ)
            gTf = cst.tile([E, B_L], F32, tag="gTf")
            iTf = cst.tile([E, B_L], F32, tag="iTf")
            nc.scalar.copy(out=gTf[:], in_=gT[:, 0, :])
            nc.scalar.copy(out=iTf[:], in_=iT[:, 0, :])

            # dedicated per-supertile hid tiles; mask row (row 16) DMA'd ONCE
            # outside the loop (iteration-invariant) -- keeps the slow
            # 1-partition mask transfer off the steady-state critical path.
            def st_geom(s):
                L = caps[s]
                ga = min(512 // L, ST_G)
                banks = (ST_G + ga - 1) // ga
                return L, ga, banks, 512 * banks

            hid_tiles = []
            mb_off = 0
            for s in range(NST):
                L, ga, banks, W = st_geom(s)
                ht = cst.tile([17, W], BF16, tag=f"hid{s}")
                nc.sync.dma_start(out=ht[16:17, :],
                                  in_=maskb[0:1, mb_off:mb_off + W])
                hid_tiles.append(ht)
                mb_off += W

            # core-wide softmax stats, consumed by the single batched tail
            den_all = cst.tile([128, B_L], F32, tag="den_all")
            pooled_all = cst.tile([128, B_L], F32, tag="pooled_all")

            # ---- per-supertile pipeline ----
            # bank grid: groups per 512-col PSUM bank; matmul outs bank-
            # aligned and <=512 wide; relu/exp batched over 2-bank chunks.
            def supertile(s):
                L, ga, banks, W = st_geom(s)
                tb = sum(caps[:s])        # tile base
                rows = 128 * L            # compact mT cols
                mT = mTp.tile([128, 1, rows], BF16)
                # split tiles across the 2 queues
                h = (L + 1) // 2
                nc.gpsimd.dma_gather(
                    out_ap=mT[:, :, 0:128 * h], in_ap=user[:],
                    idxs_ap=midx16_sb[:, 8 * tb:8 * (tb + h)],
                    num_idxs=128 * h, num_idxs_reg=128 * h, elem_size=E,
                    transpose=True, single_packet=False, queue_num=0)
                if L > h:
                    nc.gpsimd.dma_gather(
                        out_ap=mT[:, :, 128 * h:rows], in_ap=user[:],
                        idxs_ap=midx16_sb[:, 8 * (tb + h):8 * (tb + L)],
                        num_idxs=128 * (L - h), num_idxs_reg=128 * (L - h),
                        elem_size=E,
                        transpose=True, single_packet=False,
                        queue_num=1 % NQ)
                mTv = mT[:, 0, :]
                hid = hid_tiles[s]
                gst = ST_G * s            # supertile's first group
                nch = (banks + 1) // 2
                for j in range(nch):
                    b0 = 2 * j
                    nbk = min(2, banks - b0)
                    spans = []   # (local_off, gbase, gn, cw)
                    for b in range(b0, b0 + nbk):
                        gbase = ga * b
                        gn = min(ga, ST_G - gbase)
                        spans.append(((b - b0) * 512, gbase, gn, gn * L))
                    ospan = spans[-1][0] + spans[-1][3]
                    co = 512 * b0            # chunk col offset in hid space
                    hps = hpsp.tile([16, CH], F32, space="PSUM", tag="hps")
                    for (o, gbase, gn, cw) in spans:
                        ro = gbase * L
                        g0 = ST_G * s + gbase
                        nc.tensor.matmul(out=hps[:, o:o + cw], lhsT=w1a_sb[:],
                                         rhs=mTv[:, ro:ro + cw],
                                         start=True, stop=False)
                        ip_view = (iT[:, 0, g0:g0 + gn]
                                   .unsqueeze(2).to_broadcast([E, gn, L]))
                        nc.tensor.matmul(out=hps[:, o:o + cw], lhsT=w1b_sb[:],
                                         rhs=ip_view, start=False, stop=True)
                    nc.scalar.activation(
                        out=hid[0:16, co:co + ospan], in_=hps[:, 0:ospan],
                        func=AF.Relu, bias=b1p_sb[:, 0:1])
                    srp = srpp.tile([128, CH], F32, space="PSUM", tag="srp")
                    for (o, gbase, gn, cw) in spans:
                        nc.tensor.matmul(out=srp[:, o:o + cw], lhsT=w2m_sb[:],
                                         rhs=hid[:, co + o:co + o + cw],
                                         start=True, stop=True)
                    ex = exp_.tile([128, CH], BF16, tag="ex")
                    nc.scalar.activation(out=ex[:, 0:ospan],
                                         in_=srp[:, 0:ospan], func=AF.Exp)
                    prod = exp_.tile([128, CH], BF16, tag="prod")
                    paired = (len(spans) == 2 and spans[0][2] == spans[1][2])
                    if paired:
                        # one TT + one 4D reduce across both banks (bank
                        # stride 512; pad cols excluded by the ga*L extent)
                        gbase, gn, cw = spans[0][1], spans[0][2], spans[0][3]
                        ro = gbase * L
                        g0 = gst + gbase
                        bview = lambda t: (t[:, 0:CH]
                                           .rearrange("p (b x) -> p b x", b=2)
                                           [:, :, 0:cw])
                        nc.vector.tensor_tensor(
                            out=bview(prod), in0=bview(ex),
                            in1=(mTv[:, ro:ro + 2 * cw]
                                 .rearrange("p (b x) -> p b x", b=2)),
                            op=OP.mult)
                        for (src, dst) in ((ex, den_all), (prod, pooled_all)):
                            v = (src[:, 0:CH]
                                 .rearrange("p (b x) -> p b x", b=2)
                                 [:, :, 0:cw]
                                 .rearrange("p b (g m) -> p b g m", m=L))
                            nc.vector.tensor_reduce(
                                out=dst[:, g0:g0 + 2 * gn],
                                in_=v, axis=mybir.AxisListType.X, op=OP.add)
                    else:
                        for (o, gbase, gn, cw) in spans:
                            ro = gbase * L
                            g0 = gst + gbase
                            nc.vector.tensor_tensor(out=prod[:, o:o + cw],
                                                    in0=ex[:, o:o + cw],
                                                    in1=mTv[:, ro:ro + cw],
                                                    op=OP.mult)
                            nc.vector.tensor_reduce(
                                out=den_all[:, g0:g0 + gn],
                                in_=ex[:, o:o + cw].rearrange(
                                    "p (g m) -> p g m", m=L),
                                axis=mybir.AxisListType.X, op=OP.add)
                            nc.vector.tensor_reduce(
                                out=pooled_all[:, g0:g0 + gn],
                                in_=prod[:, o:o + cw].rearrange(
                                    "p (g m) -> p g m", m=L),
                                axis=mybir.AxisListType.X, op=OP.add)

            def tail():
                # one batched tail over all 1024 groups: fewer fixed costs,
                # one Sigmoid act-table load, one output DMA
                dre = smlp.tile([128, B_L], F32, tag="dre")
                nc.vector.reciprocal(out=dre[:], in_=den_all[:])
                gv = smlp.tile([128, B_L], F32, tag="gv")
                nc.vector.tensor_tensor(out=gv[:], in0=pooled_all[:],
                                        in1=dre[:], op=OP.mult)
                nc.vector.tensor_tensor(out=gv[:], in0=gv[:], in1=gTf[:],
                                        op=OP.add)
                el = smlp.tile([128, B_L], F32, tag="el")
                nc.vector.tensor_tensor(out=el[:], in0=gv[:], in1=iTf[:],
                                        op=OP.mult)
                h2s = smlp.tile([16, B_L], F32, tag="h2s")
                for o in (0, 512):
                    h2 = hpsp.tile([16, CH], F32, space="PSUM", tag="hps")
                    nc.tensor.matmul(out=h2[:, 0:512], lhsT=p1a_sb[:],
                                     rhs=el[:, o:o + 512],
                                     start=True, stop=False)
                    nc.tensor.matmul(out=h2[:, 0:512], lhsT=p1b_sb[:],
                                     rhs=gv[:, o:o + 512],
                                     start=False, stop=False)
                    nc.tensor.matmul(out=h2[:, 0:512], lhsT=p1c_sb[:],
                                     rhs=iTf[:, o:o + 512],
                                     start=False, stop=True)
                    nc.scalar.activation(out=h2s[:, o:o + 512],
                                         in_=h2[:, 0:512],
                                         func=AF.Relu, bias=p1_sb[:, 0:1])
                ot = smlp.tile([1, B_L], F32, tag="ot")
                for o in (0, 512):
                    ops = srpp.tile([128, CH], F32, space="PSUM", tag="srp")
                    nc.tensor.matmul(out=ops[0:1, 0:512], lhsT=p2m_sb[:],
                                     rhs=h2s[:, o:o + 512],
                                     start=True, stop=True)
                    nc.scalar.activation(out=ot[:, o:o + 512],
                                         in_=ops[0:1, 0:512],
                                         func=AF.Sigmoid, bias=p2v_sb[:, 0:1])
                nc.sync.dma_start(out=out[:], in_=ot[:])

            def body():
                for s in range(NST):
                    supertile(s)
                tail()

            import contextlib
            lp = (nc.allow_low_precision(reason="bf16 softmax reduces")
                  if LOWP_REDUCE else contextlib.nullcontext())
            with lp:
                if loop_K:
                    with tc.For_i(0, loop_K, 1):
                        body()
                else:
                    body()
    nc.compile()
    return nc


def caps_ok(lengths, caps):
    for c in range(8):
        l = np.sort(np.asarray(lengths)[c * B_L:(c + 1) * B_L])[::-1] + 1
        for s, cap in enumerate(caps):
            if l[ST_G * s:ST_G * (s + 1)].max() > cap:
                return False
    return True


def data_caps(lengths):
    """Exact per-supertile caps = max over cores of each sorted-rank band."""
    lengths = np.asarray(lengths)
    caps = []
    for s in range(NST):
        mx = 1
        for c in range(8):
            l = np.sort(lengths[c * B_L:(c + 1) * B_L])[::-1] + 1
            mx = max(mx, int(l[ST_G * s:ST_G * (s + 1)].max()))
        caps.append(mx)
    return tuple(caps)


def _wrap16(flat):
    return np.ascontiguousarray(
        np.tile(flat.astype(np.int16).reshape(-1, 16).T, (8, 1)))


def prep_core_inputs(user_emb, item_emb, group_emb, W1, b1, W2,
                     P1, p1, P2, p2, groups_c, items_c, member_idx_c,
                     lengths_c, caps=CAPS):
    """Host-side prep of one core's in_map (b2 dropped: softmax-invariant).

    Groups sorted by length desc; supertile s keeps caps[s] member slots per
    group. ALL member gathers go through int16-indexed dma_gather, so the
    user table is permuted to put every gathered row id below 32768."""
    f32 = np.float32
    bf16 = ml_dtypes.bfloat16
    order = np.argsort(-lengths_c, kind="stable")
    groups_c = np.asarray(groups_c)[order]
    items_c = np.asarray(items_c)[order]
    member_idx_c = np.asarray(member_idx_c)[order]
    lengths_c = np.asarray(lengths_c)[order]
    mask_parts = []
    flat_ids = []
    for s, L in enumerate(caps):
        mi = member_idx_c[ST_G * s:ST_G * (s + 1), :L].astype(np.int64)
        flat_ids.append(mi.reshape(-1))
        le = lengths_c[ST_G * s:ST_G * (s + 1)]
        mg = np.where(np.arange(L)[None, :] <= le[:, None],
                      0.0, MASK_NEG).astype(f32)          # [128, L]
        # padded 512-col bank layout: ga groups per bank, pad to 512
        ga = min(512 // L, ST_G)
        banks = (ST_G + ga - 1) // ga
        mp = np.full((banks, 512), MASK_NEG, f32)
        for b in range(banks):
            gn = min(ga, ST_G - ga * b)
            mp[b, :gn * L] = mg[ga * b:ga * b + gn].reshape(-1)
        mask_parts.append(mp.reshape(-1))
    maskb = np.concatenate(mask_parts).reshape(1, -1).astype(bf16)
    all_ids = np.concatenate(flat_ids)
    gl_ids = np.unique(all_ids)
    assert gl_ids.size <= 32768, gl_ids.size
    nu = user_emb.shape[0]
    in_gl = np.zeros(nu, bool)
    in_gl[gl_ids] = True
    perm = np.concatenate([gl_ids, np.nonzero(~in_gl)[0]]).astype(np.int64)
    inv = np.empty(nu, np.int32)
    inv[perm] = np.arange(nu, dtype=np.int32)
    user_perm = np.ascontiguousarray(
        np.asarray(user_emb, dtype=f32)[perm].astype(bf16))
    midx16 = _wrap16(inv[all_ids])
    gidx16 = _wrap16(groups_c.astype(np.int64))   # NG=20000 < 32768
    it_ids = np.unique(items_c)
    ni = item_emb.shape[0]
    in_it = np.zeros(ni, bool)
    in_it[it_ids] = True
    iperm = np.concatenate([it_ids, np.nonzero(~in_it)[0]]).astype(np.int64)
    iinv = np.empty(ni, np.int32)
    iinv[iperm] = np.arange(ni, dtype=np.int32)
    item_perm = np.ascontiguousarray(
        np.asarray(item_emb, dtype=f32)[iperm].astype(bf16))
    iidx16 = _wrap16(iinv[items_c.astype(np.int64)])
    W2m = np.concatenate([np.tile(np.asarray(W2, dtype=f32).reshape(16, 1),
                                  (1, 128)),
                          np.ones((1, 128), f32)], axis=0)
    return {
        "user_emb": user_perm,
        "group_emb": np.ascontiguousarray(np.asarray(group_emb, f32)
                                          .astype(bf16)),
        "item_emb": item_perm,
        "midx16": midx16, "gidx16": gidx16, "iidx16": iidx16,
        "maskb": np.ascontiguousarray(maskb),
        "W1a": np.ascontiguousarray(W1[:E]).astype(bf16),
        "W1b": np.ascontiguousarray(W1[E:]).astype(bf16),
        "W2m": np.ascontiguousarray(W2m).astype(bf16),
        "b1p": np.asarray(b1, dtype=f32).reshape(16, 1),
        "P1a": np.ascontiguousarray(P1[:E], dtype=f32),
        "P1b": np.ascontiguousarray(P1[E:2 * E], dtype=f32),
        "P1c": np.ascontiguousarray(P1[2 * E:], dtype=f32),
        "p1v": np.asarray(p1, dtype=f32).reshape(16, 1),
        "P2": np.ascontiguousarray(np.asarray(P2), dtype=f32),
        "p2v": np.asarray(p2, dtype=f32).reshape(1, 1),
    }, order


def prep_in_maps(inputs, caps=CAPS):
    maps, orders = [], []
    for c in range(8):
        sl = slice(c * B_L, (c + 1) * B_L)
        m, order = prep_core_inputs(
            np.asarray(inputs["user_emb"]), np.asarray(inputs["item_emb"]),
            np.asarray(inputs["group_emb"]),
            np.asarray(inputs["W1"]), np.asarray(inputs["b1"]),
            np.asarray(inputs["W2"]),
            np.asarray(inputs["P1"]), np.asarray(inputs["p1"]),
            np.asarray(inputs["P2"]), np.asarray(inputs["p2"]),
            np.asarray(inputs["groups"])[sl], np.asarray(inputs["items"])[sl],
            np.asarray(inputs["member_idx"])[sl],
            np.asarray(inputs["lengths"])[sl], caps=caps)
        maps.append(m)
        orders.append(order)
    return maps, orders


def assemble_output(results, orders):
    outs = []
    for c in range(8):
        o = np.empty(B_L, np.float32)
        o[orders[c]] = results[c]["out"].reshape(B_L)
        outs.append(o)
    return np.concatenate(outs).reshape(-1, 1)


# ---------------------------------------------------------------------------
# Self-contained entrypoint: kernel(**inputs) -> np.ndarray [8192, 1]
# ---------------------------------------------------------------------------
_NC_CACHE = {}


def _get_nc(caps):
    if caps not in _NC_CACHE:
        _NC_CACHE[caps] = build_kernel(num_devices=8, caps=caps)
    return _NC_CACHE[caps]


def kernel(**inputs) -> np.ndarray:
    from concourse.bass_utils import run_bass_kernel_spmd

    caps = data_caps(np.asarray(inputs["lengths"]))
    nc = _get_nc(caps)
    in_maps, orders = prep_in_maps(inputs, caps=caps)
    res = run_bass_kernel_spmd(nc, in_maps, core_ids=list(range(8)))
    return assemble_output(res.results, orders).astype(np.float32)
